# revision 35
# baseline (speedup 1.0000x reference)
"""Block-diagonal MLP kernel for TRN2, 8 NeuronCores.

Computes out = x @ tanh(blocks * mask) where blocks is 4096x4096 with 16
diagonal 256x256 blocks (mask is the fixed block-diagonal pattern). Only the
diagonal blocks matter (tanh(0)=0):

    out[:, 256k:256(k+1)] = x[:, 256k:256(k+1)] @ tanh(B_k)

Sharding: block-parallel. Core c owns blocks 2c and 2c+1 (512 contiguous
k/n-columns) and streams all 8192 rows of x:

    outT_shard[n, m] = sum_k b[k, n] * xT_shard[k, m]      (n, k local to core)

Wire formats (DMA is the co-bottleneck with the PE): x ships bf16 pre-scaled
by 1/8 on the host (exact: exponent shift); the output ships as float8 e3m4
holding out/8 (|out/8| < 7.7 vs e3m4 max 15.5; host decodes *8), halving the
store stream vs bf16. Weights ship host-packed in the exact SBUF lhsT layout
as bf16 (one contiguous 0.26MB DMA, first on the ring) and tanh runs
on-device in small chunks so the first LDWEIGHTS issues as early as possible.

Schedule: m-half outer, then blk, then 1024-col PSUM tiles (2 banks each,
4-tile rotation = all 8 banks) with ncol INNERMOST so each x chunk feeds two
PSUM tiles before the next chunk is needed (halves the early-stream wire
demand; A/B-verified). Each PSUM tile takes 4 matmuls (kc 0/1 accumulation x
2 chunks of 512 — 512 is the hard matmul free-dim limit, one fp32 PSUM bank)
and is evacuated whole by DVE and ACT alternating per tile — the deep
rotation keeps PSUM recycling off the PE's critical path (a 2-deep rotation
measured ~0.8us PE stalls per store boundary). The first two x tiles are
split into 1024-col chunks so the first matmul isn't gated on a full 1MB
load. Loads own the Sync HWDGE ring; stores go on the ACT ring (the only
other HWDGE ring) right after the producing evac — a dma config blocks its
sequencer until the source semaphores fire, so stores anywhere else stall
that queue. The final store is split in two so the drain tail is short.

Measured rel_l2 vs f32 reference: 1.354e-2 (dominated by the e3m4 output
rounding; gate is 2e-2). ~50-52us on hardware vs the 62.7us staged baseline.
_build_nc's keyword params are A/B probes; the defaults are the
measured-best configuration.
"""

import ml_dtypes
import numpy as np

import concourse.mybir as mybir
import concourse.tile as tile
from concourse import bacc
from concourse.bass_utils import run_bass_kernel_spmd

N_CORES = 8
N_ROWS = 8192            # rows of x / out
D = 4096                 # layer size
BLOCK = 256              # block size
BLOCKS_PER_CORE = 2      # 16 blocks / 8 cores
K_PER_CORE = BLOCKS_PER_CORE * BLOCK   # 512 k (and n) columns per core
MM_FREE = 512            # matmul moving free dim (one fp32 PSUM bank)
MSEG = 1024              # PSUM tile free dim (2 banks)
HALF = 4096              # store granularity / big x-tile free dim
OUT_SCALE = 8.0          # host folds x/8 in, decodes out*8

_nc_cache = {}


def _build_nc(ncol_inner=True, split_first=True, store_chunks=False,
              mm_free=MM_FREE, split_h1=False, defer_stores=True):
    f32 = mybir.dt.float32
    bf16 = mybir.dt.bfloat16
    e3 = mybir.dt.float8e3

    # Bacc (not Bass): its compile() runs move_matmul_waits_to_ldweights and
    # generate_event_semaphores (splits multi-sem waits down to the 1
    # sync-wait-per-instruction the hardware supports).
    nc = bacc.Bacc("TRN2")
    xT = nc.dram_tensor("xT", [K_PER_CORE, N_ROWS], bf16, kind="ExternalInput")
    wp = nc.dram_tensor("wp", [128, 1024], bf16, kind="ExternalInput")
    outT = nc.dram_tensor("outT", [K_PER_CORE, N_ROWS], e3, kind="ExternalOutput")

    with tile.TileContext(nc) as tc:
        with (
            tc.tile_pool(name="bpool", bufs=1) as bpool,
            tc.tile_pool(name="xpool", bufs=14) as xpool,
            tc.tile_pool(name="opool", bufs=6 if defer_stores else 4) as opool,
            tc.tile_pool(name="pspool", bufs=4, space="PSUM") as pspool,
        ):
            # (A PE-warmup with dummy matmuls was tried and measured ~1.5us
            # WORSE: the warmup stream runs at the mid p-state, and any
            # sub-us join gap to the real stream resets the clock ramp
            # anyway, while leftover warmup matmuls delay the real stream.)

            # --- weights: ONE contiguous DMA of the host-packed lhsT
            # layout; col chunk (blk*2+kc)*256 + n holds
            # blocks[k0+blk*256+kc*128+p, k0+blk*256+n]. First entry on the
            # Sync ring. (Measured dead ends: splitting w into two Sync
            # DMAs — the tanh's semaphore wait coarsens to later ring
            # entries, ~1.2us worse; loading w via the GpSimd SWDGE ring —
            # arrives ~2.4us later than Sync HWDGE.)
            w_raw = bpool.tile([128, 1024], bf16, name="w_raw")
            b_mm = bpool.tile([128, 1024], bf16, name="b_mm")
            nc.sync.dma_start(out=w_raw[:], in_=wp[:])

            # --- x loads on the Sync ring, in consumption order. The first
            # two tiles (q0/q1, first m-half) are split into 1024-col chunks
            # so the first matmuls aren't gated on a 1MB wire transfer.
            # (Loading early x chunks via the ACT HWDGE ring measured ~4us
            # LATE arrivals — that ring gets poor DMA-engine service while
            # the Sync ring has a backlog — so everything stays on Sync.)
            xts = {}

            def load(q, c0, c1, key):
                t = xpool.tile([128, c1 - c0], bf16, name=f"x{key}", tag="xt")
                nc.sync.dma_start(
                    out=t[:], in_=xT[q * 128:(q + 1) * 128, c0:c1]
                )
                xts[key] = t

            if split_first:
                for c in range(4):
                    load(0, c * MSEG, (c + 1) * MSEG, f"0s{c}")
                    load(1, c * MSEG, (c + 1) * MSEG, f"1s{c}")
            else:
                load(0, 0, HALF, "0h0")
                load(1, 0, HALF, "1h0")
            load(2, 0, HALF, "2h0")
            load(3, 0, HALF, "3h0")
            if split_h1:
                # q0/q1 h1 halves arrive just-in-time for the h0->h1
                # transition; 2048-col chunks land the first half earlier
                for p in range(2):
                    load(0, HALF + p * 2048, HALF + (p + 1) * 2048, f"0t{p}")
                    load(1, HALF + p * 2048, HALF + (p + 1) * 2048, f"1t{p}")
                for q in (2, 3):
                    load(q, HALF, N_ROWS, f"{q}h1")
            else:
                for q in range(4):
                    load(q, HALF, N_ROWS, f"{q}h1")

            def xslice(q, mlo):
                # [mlo, mlo+mm_free) never straddles a tile boundary
                if split_first and q < 2 and mlo < HALF:
                    t = xts[f"{q}s{mlo // MSEG}"]
                    return t[:, mlo % MSEG:mlo % MSEG + mm_free]
                if split_h1 and q < 2 and mlo >= HALF:
                    t = xts[f"{q}t{(mlo - HALF) // 2048}"]
                    return t[:, (mlo - HALF) % 2048:(mlo - HALF) % 2048 + mm_free]
                t = xts[f"{q}h{mlo // HALF}"]
                return t[:, mlo % HALF:mlo % HALF + mm_free]

            # tanh in dependency order: the (blk0, kc0/kc1, ncol0) lhsT
            # chunks first so LDWEIGHTS can start right after the table load
            for c0, c1 in [(0, 128), (256, 384), (128, 256), (384, 512),
                           (512, 1024)]:
                nc.scalar.activation(
                    b_mm[:, c0:c1], w_raw[:, c0:c1],
                    mybir.ActivationFunctionType.Tanh,
                )

            # --- matmul phases: m-half outer so the load stream keeps pace.
            # ncol_inner puts ncol inside the mseg loop so each x chunk is
            # consumed twice (ncol 0/1 share rhs data) before the next
            # chunk is needed — halves the early-stream wire-demand rate.
            if ncol_inner:
                seq = [(h, blk, m4, n) for h in range(2)
                       for blk in range(BLOCKS_PER_CORE)
                       for m4 in range(HALF // MSEG) for n in range(2)]
            else:
                seq = [(h, blk, m4, n) for h in range(2)
                       for blk in range(BLOCKS_PER_CORE)
                       for n in range(2) for m4 in range(HALF // MSEG)]
            osbs = {}
            n_last = len(seq) - 1
            # defer_stores: emit each group's store configs ~2 compute
            # groups later, so the 4.2MB of store wire doesn't compete with
            # the h1 x loads for DMA engines mid-kernel (measured: q1h1
            # arriving ~8us late -> PE stall at the h0->h1 transition).
            # Group g's osbs complete at ms 8g+6/8g+7; flush points sit
            # inside groups g+2 (g+1.5 for the penultimate group).
            flush_at = {16: [], 24: [], 28: []}
            flush_of_group = {0: 16, 1: 24, 2: 28}
            for ms_idx, (h, blk, m4, ncol) in enumerate(seq):
                if defer_stores and ms_idx in flush_at:
                    # flush on the idle GpSimd SWDGE ring: inserting these
                    # configs on the ACT queue delayed evac dispatch
                    # (measured +4us); gpsimd's queue has nothing else
                    for r0_d, h_d, osb_d in flush_at[ms_idx]:
                        nc.gpsimd.dma_start(
                            out=outT[r0_d:r0_d + 128,
                                     h_d * HALF:(h_d + 1) * HALF],
                            in_=osb_d[:],
                        )
                okey = (h, blk, ncol)
                if okey not in osbs:
                    osbs[okey] = opool.tile([128, HALF], e3, name="osb",
                                            tag="osb")
                osb = osbs[okey]
                mlo0 = h * HALF + m4 * MSEG
                ps = pspool.tile([128, MSEG], f32, name="ps")
                for kc in range(2):
                    lcol = (blk * 2 + kc) * 256 + ncol * 128
                    lhsT = b_mm[:, lcol:lcol + 128]
                    for mi in range(MSEG // mm_free):
                        nc.tensor.matmul(
                            ps[:, mi * mm_free:(mi + 1) * mm_free],
                            lhsT=lhsT,
                            rhs=xslice(blk * 2 + kc, mlo0 + mi * mm_free),
                            start=(kc == 0),
                            stop=(kc == 1),
                        )
                # evac, DVE/ACT alternating per tile: 4-deep PSUM rotation
                # keeps recycling off the PE path. The final tile splits its
                # evac across both engines so the drain tail is short.
                dst = osb[:, m4 * MSEG:(m4 + 1) * MSEG]
                if ms_idx < n_last:
                    if ms_idx % 2 == 0:
                        nc.vector.tensor_copy(dst, ps[:])
                    else:
                        nc.scalar.copy(dst, ps[:])
                else:
                    hm = MSEG // 2
                    nc.vector.tensor_copy(dst[:, 0:hm], ps[:, 0:hm])
                    nc.scalar.copy(dst[:, hm:MSEG], ps[:, hm:MSEG])
                # stores on the ACT HWDGE ring: a dma config blocks its
                # sequencer until the data's semaphores have fired, so it
                # must sit on the queue whose preceding op IS the producer
                # (the evac COPY). Stores on the Sync ring measured much
                # worse — the Sync sequencer stalls on each store's evac
                # wait. The final store is split so the drain tail after
                # the last evac is short.
                r0 = blk * BLOCK + ncol * 128
                if store_chunks:
                    if m4 % 2 == 1:
                        c0 = h * HALF + (m4 - 1) * MSEG
                        nc.scalar.dma_start(
                            out=outT[r0:r0 + 128, c0:c0 + 2 * MSEG],
                            in_=osb[:, (m4 - 1) * MSEG:(m4 + 1) * MSEG],
                        )
                elif m4 == HALF // MSEG - 1:
                    if ms_idx < n_last:
                        gidx = h * BLOCKS_PER_CORE + blk
                        if defer_stores and gidx in flush_of_group:
                            flush_at[flush_of_group[gidx]].append(
                                (r0, h, osb))
                        else:
                            nc.scalar.dma_start(
                                out=outT[r0:r0 + 128,
                                         h * HALF:(h + 1) * HALF],
                                in_=osb[:],
                            )
                    else:
                        for s in range(2):
                            nc.scalar.dma_start(
                                out=outT[r0:r0 + 128,
                                         h * HALF + s * 2 * MSEG:
                                         h * HALF + (s + 1) * 2 * MSEG],
                                in_=osb[:, s * 2 * MSEG:(s + 1) * 2 * MSEG],
                            )
    nc.compile()
    return nc


def _get_nc(**kw):
    key = tuple(sorted(kw.items()))
    if key not in _nc_cache:
        _nc_cache[key] = _build_nc(**kw)
    return _nc_cache[key]


def _make_in_maps(x, blocks):
    bf = ml_dtypes.bfloat16
    # x/8 is exact in bf16 (exponent shift); the device computes out/8 so the
    # e3m4 output wire never saturates (|out/8| < 7.7 vs e3m4 max 15.5)
    xT = (x.T / OUT_SCALE).astype(bf)  # [4096, 8192]
    in_maps = []
    for c in range(N_CORES):
        k0 = c * K_PER_CORE
        cols = []
        for blk in range(BLOCKS_PER_CORE):
            for kc in range(2):
                rlo = k0 + blk * BLOCK + kc * 128
                cols.append(blocks[rlo:rlo + 128,
                                   k0 + blk * BLOCK:k0 + (blk + 1) * BLOCK])
        wpk = np.ascontiguousarray(np.concatenate(cols, axis=1)).astype(bf)
        in_maps.append({
            "xT": np.ascontiguousarray(xT[k0:k0 + K_PER_CORE]),
            "wp": wpk,
        })
    return in_maps


def _run(x, blocks, **spmd_kwargs):
    res = run_bass_kernel_spmd(
        _get_nc(), _make_in_maps(x, blocks), core_ids=list(range(N_CORES)),
        **spmd_kwargs,
    )
    out = np.empty((N_ROWS, D), np.float32)
    for c in range(N_CORES):
        shard = res.results[c]["outT"].astype(np.float32) * OUT_SCALE
        out[:, c * K_PER_CORE:(c + 1) * K_PER_CORE] = shard.T
    return out, res


def kernel(x, blocks, mask=None):
    out, _ = _run(np.asarray(x), np.asarray(blocks))
    return out


# revision 37
# speedup vs baseline: 1.1218x; 1.1218x over previous
"""Block-diagonal MLP kernel for TRN2, 8 NeuronCores.

Computes out = x @ tanh(blocks * mask) where blocks is 4096x4096 with 16
diagonal 256x256 blocks (mask is the fixed block-diagonal pattern). Only the
diagonal blocks matter (tanh(0)=0):

    out[:, 256k:256(k+1)] = x[:, 256k:256(k+1)] @ tanh(B_k)

Sharding: block-parallel. Core c owns blocks 2c and 2c+1 (512 contiguous
k/n-columns) and streams all 8192 rows of x:

    outT_shard[n, m] = sum_k b[k, n] * xT_shard[k, m]      (n, k local to core)

Wire formats (DMA is the co-bottleneck with the PE): x ships bf16 pre-scaled
by 1/8 on the host (exact: exponent shift); the output ships as float8 e3m4
holding out/8 (|out/8| < 7.7 vs e3m4 max 15.5; host decodes *8), halving the
store stream vs bf16. Weights ship host-packed in the exact SBUF lhsT layout
as bf16 (one contiguous 0.26MB DMA, first on the ring) and tanh runs
on-device in small chunks so the first LDWEIGHTS issues as early as possible.

Schedule: m-half outer, then blk, then 1024-col PSUM tiles (2 banks each,
4-tile rotation = all 8 banks) with ncol INNERMOST so each x chunk feeds two
PSUM tiles before the next chunk is needed (halves the early-stream wire
demand; A/B-verified). Each PSUM tile takes 4 matmuls (kc 0/1 accumulation x
2 chunks of 512 — 512 is the hard matmul free-dim limit, one fp32 PSUM bank)
and is evacuated whole by DVE and ACT alternating per tile — the deep
rotation keeps PSUM recycling off the PE's critical path (a 2-deep rotation
measured ~0.8us PE stalls per store boundary). The first two x tiles are
split into 1024-col chunks so the first matmul isn't gated on a full 1MB
load. Loads own the Sync HWDGE ring; stores go on the ACT ring (the only
other HWDGE ring) right after the producing evac — a dma config blocks its
sequencer until the source semaphores fire, so stores anywhere else stall
that queue. The final store is split in two so the drain tail is short.

Measured rel_l2 vs f32 reference: 1.354e-2 (dominated by the e3m4 output
rounding; gate is 2e-2). ~50-52us on hardware vs the 62.7us staged baseline.
_build_nc's keyword params are A/B probes; the defaults are the
measured-best configuration.
"""

import ml_dtypes
import numpy as np

import concourse.mybir as mybir
import concourse.tile as tile
from concourse import bacc
from concourse.bass_utils import run_bass_kernel_spmd

N_CORES = 8
N_ROWS = 8192            # rows of x / out
D = 4096                 # layer size
BLOCK = 256              # block size
BLOCKS_PER_CORE = 2      # 16 blocks / 8 cores
K_PER_CORE = BLOCKS_PER_CORE * BLOCK   # 512 k (and n) columns per core
MM_FREE = 512            # matmul moving free dim (one fp32 PSUM bank)
MSEG = 1024              # PSUM tile free dim (2 banks)
HALF = 4096              # store granularity / big x-tile free dim
OUT_SCALE = 8.0          # host folds x/8 in, decodes out*8

_nc_cache = {}


def _build_nc(ncol_inner=True, split_first=True, store_chunks=False,
              mm_free=MM_FREE, split_h1=False, defer_stores=True):
    f32 = mybir.dt.float32
    bf16 = mybir.dt.bfloat16
    e3 = mybir.dt.float8e3

    # Bacc (not Bass): its compile() runs move_matmul_waits_to_ldweights and
    # generate_event_semaphores (splits multi-sem waits down to the 1
    # sync-wait-per-instruction the hardware supports).
    nc = bacc.Bacc("TRN2")
    xT = nc.dram_tensor("xT", [K_PER_CORE, N_ROWS], bf16, kind="ExternalInput")
    wp = nc.dram_tensor("wp", [128, 1024], bf16, kind="ExternalInput")
    outT = nc.dram_tensor("outT", [K_PER_CORE, N_ROWS], e3, kind="ExternalOutput")

    with tile.TileContext(nc) as tc:
        with (
            tc.tile_pool(name="bpool", bufs=1) as bpool,
            tc.tile_pool(name="xpool", bufs=14) as xpool,
            tc.tile_pool(name="opool", bufs=6 if defer_stores else 4) as opool,
            tc.tile_pool(name="pspool", bufs=4, space="PSUM") as pspool,
        ):
            # (A PE-warmup with dummy matmuls was tried and measured ~1.5us
            # WORSE: the warmup stream runs at the mid p-state, and any
            # sub-us join gap to the real stream resets the clock ramp
            # anyway, while leftover warmup matmuls delay the real stream.)

            # --- weights: ONE contiguous DMA of the host-packed lhsT
            # layout; col chunk (blk*2+kc)*256 + n holds
            # blocks[k0+blk*256+kc*128+p, k0+blk*256+n]. First entry on the
            # Sync ring. (Measured dead ends: splitting w into two Sync
            # DMAs — the tanh's semaphore wait coarsens to later ring
            # entries, ~1.2us worse; loading w via the GpSimd SWDGE ring —
            # arrives ~2.4us later than Sync HWDGE.)
            w_raw = bpool.tile([128, 1024], bf16, name="w_raw")
            b_mm = bpool.tile([128, 1024], bf16, name="b_mm")
            nc.sync.dma_start(out=w_raw[:], in_=wp[:])

            # --- x loads on the Sync ring, in consumption order. The first
            # two tiles (q0/q1, first m-half) are split into 1024-col chunks
            # so the first matmuls aren't gated on a 1MB wire transfer.
            # (Loading early x chunks via the ACT HWDGE ring measured ~4us
            # LATE arrivals — that ring gets poor DMA-engine service while
            # the Sync ring has a backlog — so everything stays on Sync.)
            xts = {}

            def load(q, c0, c1, key):
                t = xpool.tile([128, c1 - c0], bf16, name=f"x{key}", tag="xt")
                nc.sync.dma_start(
                    out=t[:], in_=xT[q * 128:(q + 1) * 128, c0:c1]
                )
                xts[key] = t

            if split_first:
                for c in range(4):
                    load(0, c * MSEG, (c + 1) * MSEG, f"0s{c}")
                    load(1, c * MSEG, (c + 1) * MSEG, f"1s{c}")
            else:
                load(0, 0, HALF, "0h0")
                load(1, 0, HALF, "1h0")
            load(2, 0, HALF, "2h0")
            load(3, 0, HALF, "3h0")
            if split_h1:
                # q0/q1 h1 halves arrive just-in-time for the h0->h1
                # transition; 2048-col chunks land the first half earlier
                for p in range(2):
                    load(0, HALF + p * 2048, HALF + (p + 1) * 2048, f"0t{p}")
                    load(1, HALF + p * 2048, HALF + (p + 1) * 2048, f"1t{p}")
                for q in (2, 3):
                    load(q, HALF, N_ROWS, f"{q}h1")
            else:
                for q in range(3):
                    load(q, HALF, N_ROWS, f"{q}h1")
                # split only the FINAL load: its first-half semaphore fires
                # ~1.2us earlier for the same wire bytes, and being the last
                # ring entry the extra config delays nothing downstream
                # (which is what made splitting q0h1/q1h1 a net loss)
                load(3, HALF, HALF + 2048, "3t0")
                load(3, HALF + 2048, N_ROWS, "3t1")

            def xslice(q, mlo):
                # [mlo, mlo+mm_free) never straddles a tile boundary
                if split_first and q < 2 and mlo < HALF:
                    t = xts[f"{q}s{mlo // MSEG}"]
                    return t[:, mlo % MSEG:mlo % MSEG + mm_free]
                if split_h1 and q < 2 and mlo >= HALF:
                    t = xts[f"{q}t{(mlo - HALF) // 2048}"]
                    return t[:, (mlo - HALF) % 2048:(mlo - HALF) % 2048 + mm_free]
                if not split_h1 and q == 3 and mlo >= HALF:
                    t = xts[f"3t{(mlo - HALF) // 2048}"]
                    return t[:, (mlo - HALF) % 2048:(mlo - HALF) % 2048 + mm_free]
                t = xts[f"{q}h{mlo // HALF}"]
                return t[:, mlo % HALF:mlo % HALF + mm_free]

            # tanh in dependency order: the (blk0, kc0/kc1, ncol0) lhsT
            # chunks first so LDWEIGHTS can start right after the table load
            for c0, c1 in [(0, 128), (256, 384), (128, 256), (384, 512),
                           (512, 1024)]:
                nc.scalar.activation(
                    b_mm[:, c0:c1], w_raw[:, c0:c1],
                    mybir.ActivationFunctionType.Tanh,
                )

            # --- matmul phases: m-half outer so the load stream keeps pace.
            # ncol_inner puts ncol inside the mseg loop so each x chunk is
            # consumed twice (ncol 0/1 share rhs data) before the next
            # chunk is needed — halves the early-stream wire-demand rate.
            if ncol_inner:
                seq = [(h, blk, m4, n) for h in range(2)
                       for blk in range(BLOCKS_PER_CORE)
                       for m4 in range(HALF // MSEG) for n in range(2)]
            else:
                seq = [(h, blk, m4, n) for h in range(2)
                       for blk in range(BLOCKS_PER_CORE)
                       for n in range(2) for m4 in range(HALF // MSEG)]
            osbs = {}
            n_last = len(seq) - 1
            # defer_stores: emit each group's store configs ~2 compute
            # groups later, so the 4.2MB of store wire doesn't compete with
            # the h1 x loads for DMA engines mid-kernel (measured: q1h1
            # arriving ~8us late -> PE stall at the h0->h1 transition).
            # Group g's osbs complete at ms 8g+6/8g+7; flush points sit
            # inside groups g+2 (g+1.5 for the penultimate group).
            flush_at = {16: [], 24: [], 28: []}
            flush_of_group = {0: 16, 1: 24, 2: 28}
            for ms_idx, (h, blk, m4, ncol) in enumerate(seq):
                if defer_stores and ms_idx in flush_at:
                    # flush on the idle GpSimd SWDGE ring: inserting these
                    # configs on the ACT queue delayed evac dispatch
                    # (measured +4us); gpsimd's queue has nothing else
                    for r0_d, h_d, osb_d in flush_at[ms_idx]:
                        nc.gpsimd.dma_start(
                            out=outT[r0_d:r0_d + 128,
                                     h_d * HALF:(h_d + 1) * HALF],
                            in_=osb_d[:],
                        )
                okey = (h, blk, ncol)
                if okey not in osbs:
                    osbs[okey] = opool.tile([128, HALF], e3, name="osb",
                                            tag="osb")
                osb = osbs[okey]
                mlo0 = h * HALF + m4 * MSEG
                ps = pspool.tile([128, MSEG], f32, name="ps")
                for kc in range(2):
                    lcol = (blk * 2 + kc) * 256 + ncol * 128
                    lhsT = b_mm[:, lcol:lcol + 128]
                    for mi in range(MSEG // mm_free):
                        nc.tensor.matmul(
                            ps[:, mi * mm_free:(mi + 1) * mm_free],
                            lhsT=lhsT,
                            rhs=xslice(blk * 2 + kc, mlo0 + mi * mm_free),
                            start=(kc == 0),
                            stop=(kc == 1),
                        )
                # evac, DVE/ACT alternating per tile: 4-deep PSUM rotation
                # keeps recycling off the PE path. The final tile splits its
                # evac across both engines so the drain tail is short.
                dst = osb[:, m4 * MSEG:(m4 + 1) * MSEG]
                if ms_idx < n_last:
                    if ms_idx % 2 == 0:
                        nc.vector.tensor_copy(dst, ps[:])
                    else:
                        nc.scalar.copy(dst, ps[:])
                else:
                    hm = MSEG // 2
                    nc.vector.tensor_copy(dst[:, 0:hm], ps[:, 0:hm])
                    nc.scalar.copy(dst[:, hm:MSEG], ps[:, hm:MSEG])
                # stores on the ACT HWDGE ring: a dma config blocks its
                # sequencer until the data's semaphores have fired, so it
                # must sit on the queue whose preceding op IS the producer
                # (the evac COPY). Stores on the Sync ring measured much
                # worse — the Sync sequencer stalls on each store's evac
                # wait. The final store is split so the drain tail after
                # the last evac is short.
                r0 = blk * BLOCK + ncol * 128
                if store_chunks:
                    if m4 % 2 == 1:
                        c0 = h * HALF + (m4 - 1) * MSEG
                        nc.scalar.dma_start(
                            out=outT[r0:r0 + 128, c0:c0 + 2 * MSEG],
                            in_=osb[:, (m4 - 1) * MSEG:(m4 + 1) * MSEG],
                        )
                elif m4 == HALF // MSEG - 1:
                    if ms_idx < n_last:
                        gidx = h * BLOCKS_PER_CORE + blk
                        if defer_stores and gidx in flush_of_group:
                            flush_at[flush_of_group[gidx]].append(
                                (r0, h, osb))
                        else:
                            nc.scalar.dma_start(
                                out=outT[r0:r0 + 128,
                                         h * HALF:(h + 1) * HALF],
                                in_=osb[:],
                            )
                    else:
                        for s in range(2):
                            nc.scalar.dma_start(
                                out=outT[r0:r0 + 128,
                                         h * HALF + s * 2 * MSEG:
                                         h * HALF + (s + 1) * 2 * MSEG],
                                in_=osb[:, s * 2 * MSEG:(s + 1) * 2 * MSEG],
                            )
    nc.compile()
    return nc


def _get_nc(**kw):
    key = tuple(sorted(kw.items()))
    if key not in _nc_cache:
        _nc_cache[key] = _build_nc(**kw)
    return _nc_cache[key]


def _make_in_maps(x, blocks):
    bf = ml_dtypes.bfloat16
    # x/8 is exact in bf16 (exponent shift); the device computes out/8 so the
    # e3m4 output wire never saturates (|out/8| < 7.7 vs e3m4 max 15.5)
    xT = (x.T / OUT_SCALE).astype(bf)  # [4096, 8192]
    in_maps = []
    for c in range(N_CORES):
        k0 = c * K_PER_CORE
        cols = []
        for blk in range(BLOCKS_PER_CORE):
            for kc in range(2):
                rlo = k0 + blk * BLOCK + kc * 128
                cols.append(blocks[rlo:rlo + 128,
                                   k0 + blk * BLOCK:k0 + (blk + 1) * BLOCK])
        wpk = np.ascontiguousarray(np.concatenate(cols, axis=1)).astype(bf)
        in_maps.append({
            "xT": np.ascontiguousarray(xT[k0:k0 + K_PER_CORE]),
            "wp": wpk,
        })
    return in_maps


def _run(x, blocks, **spmd_kwargs):
    res = run_bass_kernel_spmd(
        _get_nc(), _make_in_maps(x, blocks), core_ids=list(range(N_CORES)),
        **spmd_kwargs,
    )
    out = np.empty((N_ROWS, D), np.float32)
    for c in range(N_CORES):
        shard = res.results[c]["outT"].astype(np.float32) * OUT_SCALE
        out[:, c * K_PER_CORE:(c + 1) * K_PER_CORE] = shard.T
    return out, res


def kernel(x, blocks, mask=None):
    out, _ = _run(np.asarray(x), np.asarray(blocks))
    return out
